# revision 1
# baseline (speedup 1.0000x reference)
"""ConvEncoder kernel for 8 TRN2 NeuronCores (raw Bacc, manual semaphores).

Computes: emb = emb_table[x]; windows = im2col(pad(emb), WIN=5);
y = gelu(windows @ W.T + b), for x (16, 2048) int32 ids.
Sharding: data-parallel over batch - 2 batches per core x 8 cores; the
embedding table and weights are replicated per core.

Engine programs per core:
  sync:   idx/wt/bias/ident loads, then per-span output stores
  vector: idx column split, wt cast, embT halo memsets, PSUM->embT copies
  gpsimd: 32 per-128-token indirect gathers (f32 rows -> bf16 in flight)
  tensor: 32 bf16 transposes + 5 matmuls per span (PSUM f32 accumulate)
  scalar: exact GELU(+bias) PSUM->SBUF
"""

import numpy as np

import concourse.bass as bass
import concourse.mybir as mybir
from concourse import bacc
from concourse.bass import IndirectOffsetOnAxis
from concourse.bass_utils import run_bass_kernel_spmd

B, S, EMB, WIN, OUT, VOCAB = 16, 2048, 128, 5, 128, 50257
NCORES = 8
BPC = B // NCORES
T = BPC * S                    # 4096 tokens/core
NTILE = T // 128               # 32
TPB = S // 128                 # 16
SPAN = 512
NSPAN = T // SPAN              # 8
SPB = S // SPAN                # 4
HALO = WIN // 2
EC = S + 2 * HALO              # 2052

MM_DT = mybir.dt.bfloat16
NPT = 5                        # transpose psum banks
NPS = 3                        # matmul psum banks / ao buffers

# output pieces: spans 0..6 full, span 7 split in halves
# (j, col offset within span, width, last embT tile needed)
PIECES = []
for j in range(NSPAN - 1):
    need = min(4 * j + 4, ((j // SPB) + 1) * TPB - 1)
    PIECES.append((j, 0, SPAN, need))
PIECES.append((NSPAN - 1, 0, SPAN // 2, 30))
PIECES.append((NSPAN - 1, SPAN // 2, SPAN // 2, 31))
PIECE_AFTER_TILE = {}
for p, (j, off, w, need) in enumerate(PIECES):
    PIECE_AFTER_TILE.setdefault(need, []).append(p)

_cache = {}


def _build():
    nc = bacc.Bacc("TRN2", target_bir_lowering=False, debug=False)
    xi = nc.declare_dram_parameter("xi", [128, NTILE], mybir.dt.int32, isOutput=False)
    tbl = nc.declare_dram_parameter("tbl", [VOCAB, EMB], mybir.dt.float32, isOutput=False)
    wt = nc.declare_dram_parameter("wt", [128, WIN * OUT], mybir.dt.float32, isOutput=False)
    bv = nc.declare_dram_parameter("bias", [128, 1], mybir.dt.float32, isOutput=False)
    idm = nc.declare_dram_parameter("idm", [128, 128], mybir.dt.uint16, isOutput=False)
    out = nc.declare_dram_parameter("out", [128, T], mybir.dt.float32, isOutput=True)

    idx_sb = nc.alloc_sbuf_tensor("idx_sb", [128, NTILE], mybir.dt.int32)
    idxcs = [nc.alloc_sbuf_tensor(f"idxc{c}", [128, 1], mybir.dt.int32) for c in range(NTILE)]
    gb = nc.alloc_sbuf_tensor("gb", [128, NTILE, EMB], MM_DT)
    embT = nc.alloc_sbuf_tensor("embT", [128, BPC * EC], MM_DT)
    wt_f32 = nc.alloc_sbuf_tensor("wt_f32", [128, WIN * OUT], mybir.dt.float32)
    wt_sb = nc.alloc_sbuf_tensor("wt_sb", [128, WIN * OUT], MM_DT)
    b_sb = nc.alloc_sbuf_tensor("b_sb", [128, 1], mybir.dt.float32)
    ident = nc.alloc_sbuf_tensor("ident", [128, 128], MM_DT)
    aos = [nc.alloc_sbuf_tensor(f"ao{i}", [128, SPAN], mybir.dt.float32) for i in range(NPS)]
    pts = [nc.alloc_psum_tensor(f"pt{i}", [128, 128], MM_DT) for i in range(NPT)]
    pss = [nc.alloc_psum_tensor(f"ps{i}", [128, SPAN], mybir.dt.float32) for i in range(NPS)]

    with (
        nc.semaphore("s_idx") as s_idx,
        nc.semaphore("s_ld") as s_ld,
        nc.semaphore("s_idxc") as s_idxc,
        nc.semaphore("s_g") as s_g,
        nc.semaphore("s_t") as s_t,
        nc.semaphore("s_e") as s_e,
        nc.semaphore("s_wtc") as s_wtc,
        nc.semaphore("s_mm") as s_mm,
        nc.semaphore("s_act") as s_act,
        nc.semaphore("s_out") as s_out,
        nc.Block(no_gpsimd_drain=True) as block,
    ):

        @block.sync
        def _(sync):
            sync.dma_start(out=idx_sb[:], in_=xi[:]).then_inc(s_idx, 16)
            sync.dma_start(out=wt_f32[:], in_=wt[:]).then_inc(s_ld, 16)
            sync.dma_start(out=b_sb[:], in_=bv[:]).then_inc(s_ld, 16)
            sync.dma_start(
                out=ident[:].bitcast(mybir.dt.uint16), in_=idm[:]
            ).then_inc(s_ld, 16)
            for p, (j, off, w, _need) in enumerate(PIECES):
                sync.wait_ge(s_act, p + 1)
                sync.dma_start(
                    out=out[:, j * SPAN + off : j * SPAN + off + w],
                    in_=aos[p % NPS][:, 0:w],
                ).then_inc(s_out, 16)
            sync.wait_ge(s_out, 16 * len(PIECES))

        @block.vector
        def _(vector):
            for bb in range(BPC):
                nc.vector.memset(embT[:, bb * EC : bb * EC + HALO], 0.0)
                nc.vector.memset(embT[:, bb * EC + HALO + S : (bb + 1) * EC], 0.0)
            vector.wait_ge(s_idx, 16)
            for c in range(NTILE):
                nc.vector.tensor_copy(
                    out=idxcs[c][:], in_=idx_sb[:, c : c + 1]
                ).then_inc(s_idxc, 1)
            vector.wait_ge(s_ld, 16)
            nc.vector.tensor_copy(out=wt_sb[:], in_=wt_f32[:]).then_inc(s_wtc, 1)
            for c in range(NTILE):
                vector.wait_ge(s_t, c + 1)
                bb, tl = c // TPB, (c % TPB) * 128
                nc.vector.tensor_copy(
                    out=embT[:, bb * EC + HALO + tl : bb * EC + HALO + tl + 128],
                    in_=pts[c % NPT][:],
                ).then_inc(s_e, 1)

        @block.gpsimd
        def _(gpsimd):
            for c in range(NTILE):
                gpsimd.wait_ge(s_idxc, c + 1)
                nc.gpsimd.indirect_dma_start(
                    out=gb[:, c, :],
                    out_offset=None,
                    in_=tbl[:],
                    in_offset=IndirectOffsetOnAxis(ap=idxcs[c][:], axis=0),
                ).then_inc(s_g, 16)

        @block.tensor
        def _(tensor):
            tensor.wait_ge(s_ld, 48)   # identity loaded
            first_mm = True
            for c in range(NTILE):
                tensor.wait_ge(s_g, 16 * (c + 1))
                if c >= NPT:
                    tensor.wait_ge(s_e, c - NPT + 1)   # pt bank free
                nc.tensor.transpose(
                    out=pts[c % NPT][:], in_=gb[:, c, :], identity=ident[:]
                ).then_inc(s_t, 1)
                for p in PIECE_AFTER_TILE.get(c, []):
                    j, off, w, need = PIECES[p]
                    tensor.wait_ge(s_e, need + 1)
                    if first_mm:
                        tensor.wait_ge(s_wtc, 1)
                        first_mm = False
                    if p >= NPS:
                        tensor.wait_ge(s_act, p - NPS + 1)   # ps bank free
                    bb, ts0 = j // SPB, (j % SPB) * SPAN
                    ps = pss[p % NPS]
                    for k in range(WIN):
                        mm = nc.tensor.matmul(
                            out=ps[:, 0:w],
                            lhsT=wt_sb[:, k * OUT : (k + 1) * OUT],
                            rhs=embT[:, bb * EC + ts0 + off + k : bb * EC + ts0 + off + k + w],
                            start=(k == 0),
                            stop=(k == WIN - 1),
                        )
                    mm.then_inc(s_mm, 1)

        @block.scalar
        def _(scalar):
            scalar.wait_ge(s_ld, 32)   # bias loaded
            for p, (j, off, w, _need) in enumerate(PIECES):
                scalar.wait_ge(s_mm, p + 1)
                if p >= NPS:
                    scalar.wait_ge(s_out, 16 * (p - NPS + 1))   # ao buffer free
                nc.scalar.activation(
                    out=aos[p % NPS][:, 0:w],
                    in_=pss[p % NPS][:, 0:w],
                    func=mybir.ActivationFunctionType.Gelu,
                    bias=b_sb[:, 0:1],
                ).then_inc(s_act, 1)

    nc.compile()
    return nc


def _prep_inputs(x, emb_table, W, b):
    import ml_dtypes

    x = np.asarray(x).astype(np.int32)
    emb_table = np.ascontiguousarray(np.asarray(emb_table, dtype=np.float32))
    W = np.asarray(W, dtype=np.float32)
    b = np.asarray(b, dtype=np.float32)
    wt = np.ascontiguousarray(
        W.reshape(OUT, WIN, EMB).transpose(2, 1, 0).reshape(EMB, WIN * OUT)
    )
    bias = np.ascontiguousarray(b.reshape(128, 1))
    idm = np.eye(128, dtype=ml_dtypes.bfloat16).view(np.uint16)
    in_maps = []
    for core in range(NCORES):
        flat = x[core * BPC : (core + 1) * BPC].reshape(-1)
        xic = np.ascontiguousarray(flat.reshape(NTILE, 128).T)
        in_maps.append({"xi": xic, "tbl": emb_table, "wt": wt, "bias": bias, "idm": idm})
    return in_maps


def kernel(x, emb_table, W, b, _trace=False):
    if "nc" not in _cache:
        _cache["nc"] = _build()
    nc = _cache["nc"]
    in_maps = _prep_inputs(x, emb_table, W, b)
    res = run_bass_kernel_spmd(nc, in_maps, core_ids=list(range(NCORES)), trace=_trace)
    _cache["last_result"] = res
    outs = []
    for core in range(NCORES):
        oc = res.results[core]["out"]
        outs.append(oc.T.reshape(BPC, S, OUT))
    return np.concatenate(outs, axis=0)

